# revision 1
# baseline (speedup 1.0000x reference)
"""Trainium2 Bass kernel for nn_MessagePassing (gnn_message_passing).

Reference computation (2 steps):
    h    = relu(cur @ mW1 + mb1)                      # per-module MLP layer 1
    msg  = h @ mW2 + mb2                              # per-module MLP layer 2
    rec  = einsum('mn,bnd->bmd', C, msg) * w[:,:,None]
    g    = relu(concat([cur, rec], -1) @ aW1 + ab1)
    cur  = cur + g @ aW2 + ab2

Strategy (data-parallel over 8 NeuronCores, 8192 batch rows each):
  * T-layout on chip: SBUF tiles are [128 features (partitions), cols] where
    a column is one (b, m) row of the flattened [B*M, 128] stream. Every
    per-module linear layer is one weights-stationary bf16 matmul streaming
    512 columns per instruction; PSUM fp32.
  * Algebraic refactors (host-side):
      - The mW2 pass is eliminated: rec_contrib = (w ⊙ mix(h')) @ Q with
        Q = mW2 @ aW1_bot.
      - The mb2 bias term (a rank-1 s⊗qb correction) is eliminated by
        adding v = inv(mW2^T) @ mb2 to h post-relu:
        mix(h + v) @ Q picks up exactly s ⊗ qb after the w-scale. The +v
        rides the relu evacuation for free: relu(x)+v = max(x+v, v), one
        tensor_scalar with two per-partition scalar APs.
      - ab2 is folded into the step-1 residual (c1b = up0+ab2+xb); the host
        adds the final ab2. The residual base is bf16 (x enters as bf16).
  * The module mix runs on the PE as ONE weights-stationary matmul with
    W_mix = kron(I_16, C.T) over DMA-XBAR-transposed 128-col blocks.
    All transposes ride ONE DMA queue (nc.sync) exclusively.
  * Rolling two-stream software pipeline: step-0 of quad q and step-1 of
    quad q-W advance together each tick through stages
    load -> frontA (h matmul + relu+v evac + transpose) ->
    frontB (mix matmul + w-scale evac + transpose back) ->
    back (aW1 accumulate, relu, aW2, residual).
    No supergroup barriers: every engine queue stays saturated, which also
    keeps the PE at its max DVFS p-state (2.4 GHz needs >3us continuous).
  * PSUM evacuations are spread across DVE / ACT / Pool to keep all three
    below the Tensor/Sync roofline.
"""

import os
import sys

import numpy as np

try:
    import concourse.bass as bass
except ImportError:  # harness runs kernel.py from a bare directory
    sys.path.insert(0, "/opt/trn_rl_repo")
    import concourse.bass as bass

import ml_dtypes
import concourse.bacc as bacc
import concourse.mybir as mybir
from concourse.tile import TileContext

BF16 = ml_dtypes.bfloat16
D = 128
M = 8
GRP = 512
QB = 4                  # groups per quad (DMA/transpose batch)
QCOLS = QB * GRP        # 2048
NCORES = 8
NSTEPS = 2

# rolling pipeline lags (in ticks)
W_LAG = 4               # step-1 stream runs W_LAG quads behind step-0
LA = 1                  # frontA -> frontB lag
LB = 2                  # frontA -> back lag
PF = 3                  # xb load prefetch distance

_nc_cache = {}


def build_nc(cols):
    """Build (and cache) the per-core Bass program for a `cols`-wide shard."""
    if cols in _nc_cache:
        return _nc_cache[cols]
    assert cols % QCOLS == 0
    ng = cols // GRP
    nq = cols // QCOLS

    f32 = mybir.dt.float32
    bf = mybir.dt.bfloat16
    relu = mybir.ActivationFunctionType.Relu
    identf = mybir.ActivationFunctionType.Identity
    add = mybir.AluOpType.add
    mult = mybir.AluOpType.mult
    amax = mybir.AluOpType.max

    nc = bacc.Bacc(trn_type="TRN2")
    xb_d = nc.declare_dram_parameter("xb", [nq, D, QCOLS], bf, isOutput=False)
    ident_d = nc.declare_dram_parameter("ident", [D, D], bf, isOutput=False)
    wcol_d = nc.declare_dram_parameter("wcol", [D, 4 * ng], f32, isOutput=False)
    wm1_d = nc.declare_dram_parameter("wm1", [D, D], bf, isOutput=False)
    wmx_d = nc.declare_dram_parameter("wmx", [D, D], bf, isOutput=False)
    wq_d = nc.declare_dram_parameter("wq", [D, D], bf, isOutput=False)
    wa1t_d = nc.declare_dram_parameter("wa1t", [D, D], bf, isOutput=False)
    wa2_d = nc.declare_dram_parameter("wa2", [D, D], bf, isOutput=False)
    mb1v_d = nc.declare_dram_parameter("mb1v", [D, 1], f32, isOutput=False)
    vv_d = nc.declare_dram_parameter("vv", [D, 1], f32, isOutput=False)
    ab1_d = nc.declare_dram_parameter("ab1", [D, 1], f32, isOutput=False)
    ab2_d = nc.declare_dram_parameter("ab2", [D, 1], f32, isOutput=False)
    out_d = nc.declare_dram_parameter("out", [nq, D, QCOLS], bf, isOutput=True)

    with TileContext(nc) as tc:
        with (
            tc.tile_pool(name="consts", bufs=1) as cp,
            tc.tile_pool(name="work", bufs=2) as wp,
            tc.tile_pool(name="pipe", bufs=3) as fp,
            tc.tile_pool(name="psum", bufs=2, space="PSUM") as pp,
        ):
            ident = cp.tile_from(forced_dma_engine=mybir.EngineType.Pool, ap=ident_d[:, :])
            w_m1 = cp.tile_from(forced_dma_engine=mybir.EngineType.Pool, ap=wm1_d[:, :])
            w_mx = cp.tile_from(forced_dma_engine=mybir.EngineType.Pool, ap=wmx_d[:, :])
            w_q = cp.tile_from(forced_dma_engine=mybir.EngineType.Pool, ap=wq_d[:, :])
            w_a1t = cp.tile_from(forced_dma_engine=mybir.EngineType.Pool, ap=wa1t_d[:, :])
            w_a2 = cp.tile_from(forced_dma_engine=mybir.EngineType.Pool, ap=wa2_d[:, :])
            wcol = cp.tile_from(forced_dma_engine=mybir.EngineType.Pool, ap=wcol_d[:, :])
            mb1v = cp.tile_from(forced_dma_engine=mybir.EngineType.Pool, ap=mb1v_d[:, :])
            vv = cp.tile_from(forced_dma_engine=mybir.EngineType.Pool, ap=vv_d[:, :])
            ab1 = cp.tile_from(forced_dma_engine=mybir.EngineType.Pool, ap=ab1_d[:, :])
            ab2 = cp.tile_from(forced_dma_engine=mybir.EngineType.Pool, ap=ab2_d[:, :])

            xb_t = {}      # q -> bf16 input tile (step-0 operand + residual)
            c1b_t = {}     # q -> bf16 step-1 operand (x + up0 + ab2)
            hR_t = {}      # (q, s) -> transposed h
            smixT_t = {}   # (q, s) -> transposed-back scaled mix

            def load(q):
                xb_t[q] = fp.tile([D, QCOLS], bf, tag="xb", bufs=7,
                                  name=f"xb{q}")
                nc.gpsimd.dma_start(xb_t[q][:], xb_d[q])

            def frontA(q, s):
                cur = xb_t[q] if s == 0 else c1b_t[q]
                h = wp.tile([D, QCOLS], bf, tag="h", bufs=3, name="h")
                for j in range(QB):
                    cs = slice(j * GRP, (j + 1) * GRP)
                    hp = pp.tile([D, GRP], f32, tag="hp", name="hp")
                    nc.tensor.matmul(
                        hp[:], w_m1[:], cur[:, cs], start=True, stop=True
                    )
                    # h = relu(hp + mb1) + v == max(hp + (mb1+v), v)
                    # (Pool cannot read PSUM; ACT cannot apply the second
                    # per-partition scalar -> DVE only)
                    nc.vector.tensor_scalar(h[:, cs], hp[:], mb1v[:], vv[:], add, amax)
                hR_t[(q, s)] = wp.tile([D, QB * 4, D], bf, tag="hR", bufs=3,
                                       name=f"hR{s}")
                nc.sync.dma_start_transpose(hR_t[(q, s)][:], h[:])

            def frontB(q, s):
                hR = hR_t.pop((q, s))
                smix = wp.tile([D, QCOLS], bf, tag="smix", bufs=3, name="smix")
                for j in range(QB):
                    g = q * QB + j
                    cs = slice(j * GRP, (j + 1) * GRP)
                    mixp = pp.tile([D, GRP], f32, tag="mixp", name="mixp")
                    nc.tensor.matmul(
                        mixp[:], w_mx[:], hR[:, j * 4 : (j + 1) * 4, :],
                        start=True, stop=True,
                    )
                    # w-scale evac (transposed layout: w is per-partition,
                    # constant within each 128-col block)
                    if j == 1:
                        # offload one group to ACT as 4 per-block scaled
                        # copies (keeps DVE below the Tensor/DVE co-limit)
                        for t in range(4):
                            col = 4 * g + t
                            bs = slice(j * GRP + t * D, j * GRP + (t + 1) * D)
                            nc.scalar.activation(
                                smix[:, bs], mixp[:, t * D : (t + 1) * D],
                                mybir.ActivationFunctionType.Copy,
                                bias=0.0, scale=wcol[:, col : col + 1],
                            )
                    else:
                        nc.vector.tensor_tensor(
                            smix[:, cs].rearrange("a (b c) -> a b c", b=4),
                            mixp[:].rearrange("a (b c) -> a b c", b=4),
                            wcol[:, 4 * g : 4 * g + 4].broadcast_to((D, 4, D)),
                            mult,
                        )
                smixT_t[(q, s)] = fp.tile([D, QB * 4, D], bf, tag="smixT",
                                          bufs=4, name=f"sT{s}")
                nc.sync.dma_start_transpose(smixT_t[(q, s)][:], smix[:])

            def back(q, s):
                cur = xb_t[q] if s == 0 else c1b_t[q]
                sT = smixT_t.pop((q, s))
                if s == 0:
                    c1b_t[q] = fp.tile([D, QCOLS], bf, tag="c1b", bufs=7,
                                       name=f"c1b{q}")
                else:
                    onew = wp.tile([D, QCOLS], bf, tag="onew", bufs=3,
                                   name="onew")
                for j in range(QB):
                    cs = slice(j * GRP, (j + 1) * GRP)
                    gp = pp.tile([D, GRP], f32, tag="gp", name="gp")
                    nc.tensor.matmul(
                        gp[:], w_q[:], sT[:, j * 4 : (j + 1) * 4, :],
                        start=True, stop=False,
                    )
                    nc.tensor.matmul(
                        gp[:], w_a1t[:], cur[:, cs], start=False, stop=True
                    )
                    gt = wp.tile([D, GRP], bf, tag="gt", bufs=3, name="gt")
                    nc.scalar.activation(gt[:], gp[:], relu, bias=ab1[:])
                    up = pp.tile([D, GRP], f32, tag="up", name="up")
                    nc.tensor.matmul(up[:], w_a2[:], gt[:], start=True, stop=True)
                    if s == 0:
                        # u1t = up + ab2 (ACT); c1b = u1t + xb (DVE, all-bf16)
                        u1t = wp.tile([D, GRP], bf, tag="u1t", bufs=3,
                                      name="u1t")
                        nc.scalar.activation(u1t[:], up[:], identf, bias=ab2[:])
                        nc.vector.tensor_tensor(
                            c1b_t[q][:, cs], u1t[:], xb_t[q][:, cs], add
                        )
                    else:
                        nc.vector.tensor_tensor(
                            onew[:, cs], up[:], c1b_t[q][:, cs], add
                        )
                if s == 1:
                    nc.gpsimd.dma_start(out_d[q], onew[:])

            for q in range(min(PF, nq)):
                load(q)
            for t in range(nq + W_LAG + LB + 1):
                lq = t + PF
                if lq < nq:
                    load(lq)
                if t < nq:
                    frontA(t, 0)
                if 0 <= t - LA < nq:
                    frontB(t - LA, 0)
                if 0 <= t - LB < nq:
                    back(t - LB, 0)
                    del xb_t[t - LB]
                u = t - W_LAG
                if 0 <= u < nq:
                    frontA(u, 1)
                if 0 <= u - LA < nq:
                    frontB(u - LA, 1)
                if 0 <= u - LB < nq:
                    back(u - LB, 1)
                    del c1b_t[u - LB]

    nc.compile()
    _nc_cache[cols] = nc
    return nc


def host_prep(module_states, connection_matrix, module_weights,
              mW1, mb1, mW2, mb2, aW1, ab1, aW2, ab2, ncores=NCORES):
    """Shard + precompute all host-side tensors. Returns (cols, in_maps)."""
    ms = np.asarray(module_states, np.float32)
    C = np.asarray(connection_matrix, np.float32)
    w = np.asarray(module_weights, np.float32)
    mW1 = np.asarray(mW1, np.float32)
    mb1 = np.asarray(mb1, np.float32)
    mW2 = np.asarray(mW2, np.float32)
    mb2 = np.asarray(mb2, np.float32)
    aW1 = np.asarray(aW1, np.float32)
    ab1 = np.asarray(ab1, np.float32)
    aW2 = np.asarray(aW2, np.float32)
    ab2 = np.asarray(ab2, np.float32)

    B = ms.shape[0]
    bsh = B // ncores
    cols = bsh * M

    # v = inv(mW2^T) mb2: adding v to h post-relu makes the downstream
    # mix -> w-scale -> Q pipeline emit exactly the s (x) qb rank-1 term
    # that the mb2 bias would have produced.
    v = np.linalg.solve(mW2.astype(np.float64).T, mb2.astype(np.float64))
    v = v.astype(np.float32)

    consts = {
        "ident": np.eye(D, dtype=np.float32).astype(BF16),
        "wm1": mW1.astype(BF16),
        "wmx": np.kron(np.eye(16, dtype=np.float32), C.T).astype(BF16),
        "wq": (mW2 @ aW1[D:, :]).astype(BF16),
        "wa1t": np.ascontiguousarray(aW1[:D, :]).astype(BF16),
        "wa2": aW2.astype(BF16),
        "mb1v": np.ascontiguousarray((mb1 + v).reshape(D, 1)),
        "vv": np.ascontiguousarray(v.reshape(D, 1)),
        "ab1": np.ascontiguousarray(ab1.reshape(D, 1)),
        "ab2": np.ascontiguousarray(ab2.reshape(D, 1)),
    }

    ng = cols // GRP
    nq = cols // QCOLS
    in_maps = []
    for k in range(ncores):
        shard = ms[k * bsh : (k + 1) * bsh]
        xT = shard.reshape(cols, D).T                       # [128, cols]
        xb = np.ascontiguousarray(
            xT.reshape(D, nq, QCOLS).transpose(1, 0, 2)     # [nq, 128, 2048]
        ).astype(BF16)
        wk = w[k * bsh : (k + 1) * bsh]
        wflat = wk.reshape(cols)
        wcol = np.ascontiguousarray(wflat.reshape(4 * ng, D).T)
        in_maps.append({"xb": xb, "wcol": wcol, **consts})
    return cols, in_maps


def gather_out(results, ab2, ncores=NCORES):
    """Device out = bf16(x) + up0 + ab2 + up1; host adds the final ab2."""
    ab2 = np.asarray(ab2, np.float32)
    outs = []
    for k in range(ncores):
        o = np.asarray(results[k]["out"]).astype(np.float32)
        nq = o.shape[0]
        cols = nq * QCOLS
        bsh = cols // M
        oT = o.transpose(1, 0, 2).reshape(D, cols)  # [128, cols]
        outs.append(oT.T.reshape(bsh, M, D))
    out = np.concatenate(outs, 0)
    out += ab2[None, None, :]
    return out.astype(np.float32)


def _run(inputs, trace=False):
    from concourse.bass_utils import run_bass_kernel_spmd

    cols, in_maps = host_prep(**inputs)
    nc = build_nc(cols)
    res = run_bass_kernel_spmd(nc, in_maps, list(range(NCORES)), trace=trace)
    out = gather_out(res.results, inputs["ab2"])
    return out, res


def kernel(**inputs):
    out, _ = _run(inputs, trace=False)
    return out

